# revision 9
# baseline (speedup 1.0000x reference)
"""Fused attention kernel for TRN2, SPMD across 8 NeuronCores.

Problem: out = softmax(mask ? (Q Wq^T + bq)(K Wk^T + bk)^T / sqrt(D) : -1e9)
               @ (V Wv^T + bv)
with B=4, L=2048, E=D=1024.

Sharding: core c handles batch b=c//2, query-half h=c%2 (1024 query rows).
No collectives needed; K/V rows for the batch are fully loaded per core.

Algebra (per core; Xq = Q-shard (1024,E), Xk = K[b] (2048,E), Xv = V[b]):
  scores = (Xq @ Wqk) @ Xk^T + 1 (x) w^T          Wqk = Wq^T Wk
                                                  w   = Xk @ (Wk^T bq)
  (q.bk and bq.bk terms are per-query-row constants and cancel in softmax;
  the 1/sqrt(D) scale is applied at the Exp activation, keeping tT in fp8
  normal range)
  p   = exp(s/32) * mask        (unnormalized; softmax denom deferred)
  out = ((p @ Xv) @ Wv^T) * (1/sum(p)) + 1 (x) bv

Phase dtypes: phase 1 (Q proj) bf16; phase 2 (scores) fp8 e4m3 with
DoubleRow perf mode (2 contraction subtiles per matmul); phases 4/5 bf16.

Software pipeline: scores+softmax of pair k+1 are emitted before the
AV/out-proj (back) of pair k, so the ACT exp + DVE mask/denom chain of
k+1 hides under back(k)'s ~20us of PE work.
"""
from contextlib import ExitStack

import numpy as np

import concourse.bacc as bacc
import concourse.tile as tile
from concourse import mybir
from concourse.bass_utils import run_bass_kernel_spmd
from concourse.masks import make_identity

F32 = mybir.dt.float32
BF16 = mybir.dt.bfloat16
FP8 = mybir.dt.float8e4
AF = mybir.ActivationFunctionType
ALU = mybir.AluOpType
DR = mybir.MatmulPerfMode.DoubleRow

B, L, E, D = 4, 2048, 1024, 1024
LS = 1024          # query rows per core
J = 2048           # key rows per core
P = 128
NCORES = 8
SCALE = 1.0 / 32.0  # 1/sqrt(D), applied at the Exp activation

EC = E // P        # 8 chunks of 128 along E/D dims
JC = J // P        # 16 chunks along J
LT = LS // P       # 8 query tiles per core
NP = LT // 2       # 4 query-tile pairs


def _transpose_chunks(nc, ps_tr, src, dst_fn, nblk, ident, psdt, lbl,
                      dve_every=4):
    """Transpose nblk [P,P] blocks of src (groups of 4 share a psum bank).

    src: AP [P, nblk*P]; dst_fn(i) -> destination AP [P, P] for block i.
    1 in dve_every evictions go to DVE, the rest to ACT.
    """
    for t0 in range(0, nblk, 4):
        ps = ps_tr.tile([P, 512], psdt, name=f"pstr_{lbl}", tag="tr")
        for k in range(4):
            nc.tensor.transpose(
                ps[:, k * P:(k + 1) * P],
                src[:, (t0 + k) * P:(t0 + k + 1) * P],
                ident[:],
            )
        for k in range(4):
            dst = dst_fn(t0 + k)
            srcp = ps[:, k * P:(k + 1) * P]
            if (t0 // 4 + k) % dve_every == 0:
                nc.vector.tensor_copy(dst, srcp)
            else:
                nc.scalar.activation(out=dst, in_=srcp, func=AF.Copy)


def _build():
    nc = bacc.Bacc(None, target_bir_lowering=False)

    Xq_e = nc.declare_dram_parameter("XqT", [E, LS], BF16, isOutput=False)
    Xk_e = nc.declare_dram_parameter("XkT", [E, J], FP8, isOutput=False)
    Xv_e = nc.declare_dram_parameter("Xv", [J, E], BF16, isOutput=False)
    Mk_e = nc.declare_dram_parameter("mask", [LS, J], BF16, isOutput=False)
    Wqk_e = nc.declare_dram_parameter("Wqk", [E, E], BF16, isOutput=False)
    kb_e = nc.declare_dram_parameter("kb", [E], F32, isOutput=False)
    Wv_e = nc.declare_dram_parameter("WvT", [E, D], BF16, isOutput=False)
    bv_e = nc.declare_dram_parameter("bv", [D], F32, isOutput=False)
    out_e = nc.declare_dram_parameter("out", [LS, D], F32, isOutput=True)

    # chunked DRAM views: [p, chunk, free]
    XqT_d = Xq_e.ap().rearrange("(c p) l -> p c l", p=P)
    XkT_d = Xk_e.ap().rearrange("(c p) j -> p c j", p=P)
    Xv_d = Xv_e.ap().rearrange("(c p) e -> p c e", p=P)
    Wqk_d = Wqk_e.ap().rearrange("(c p) e -> p c e", p=P)
    kb_d = kb_e.ap().rearrange("(c p) -> p c", p=P)
    WvT_d = Wv_e.ap().rearrange("(c p) d -> p c d", p=P)
    Mk_d = Mk_e.ap().rearrange("(c p) j -> p c j", p=P)
    out_d = out_e.ap().rearrange("(c p) d -> p c d", p=P)

    with tile.TileContext(nc) as tc, ExitStack() as long_pools:
        lp_pool = lambda name: long_pools.enter_context(
            tc.tile_pool(name=name, bufs=1))
        with (
            tc.tile_pool(name="ps_s", bufs=2, space="PSUM") as ps_s,
            tc.tile_pool(name="ps_mm", bufs=2, space="PSUM") as ps_mm,
            tc.tile_pool(name="ps_tr", bufs=3, space="PSUM") as ps_tr,
        ):
            # ---- constants ----
            consts = lp_pool("consts")
            ident_f = consts.tile([P, P], F32, name="ident_f")
            make_identity(nc, ident_f[:])
            ident_b = consts.tile([P, P], BF16, name="ident_b")
            nc.vector.tensor_copy(ident_b[:], ident_f[:])

            bvb_sb = consts.tile([P, D], F32, name="bvb_sb")
            kb_sb = consts.tile([P, EC], F32, name="kb_sb")

            tT_sb = lp_pool("tT_p").tile([P, EC, LS], FP8, name="tT_sb")
            XkT_sb = lp_pool("XkT_p").tile([P, EC, J], FP8, name="XkT_sb")
            mask_sb = lp_pool("mask_p").tile([P, LT, J], BF16, name="mask_sb")

            # PE warmup: no-DMA transposes fill the initial DMA-latency
            # window and bring the PE out of its cold p-state before the
            # first real matmuls
            for wu in range(12):
                ps = ps_tr.tile([P, 512], F32, name="pswu", tag="tr")
                for k in range(4):
                    nc.tensor.transpose(ps[:, k * P:(k + 1) * P],
                                        ident_f[:], ident_f[:])

            # ===== stage A+B: Wqk ; kb ; XqT ; phase 1 ; XkT =====
            with (
                tc.tile_pool(name="wqk_pool", bufs=1) as wqk_pool,
                tc.tile_pool(name="xqt_pool", bufs=1) as xqt_pool,
            ):
                wqk_sb = wqk_pool.tile([P, EC, E], BF16, name="wqk_sb")
                xqT_sb = xqt_pool.tile([P, EC, LS], BF16, name="xqT_sb")
                nc.sync.dma_start(out=kb_sb[:], in_=kb_d)
                for c in range(EC):
                    nc.sync.dma_start(out=wqk_sb[:, c, :],
                                      in_=Wqk_d[:, c, :])
                    nc.scalar.dma_start(out=xqT_sb[:, c, :],
                                        in_=XqT_d[:, c, :])
                import concourse.bass as _bass
                bv_bcast = _bass.AP(tensor=bv_e, offset=0,
                                    ap=[[0, P], [1, D]])
                nc.scalar.dma_start(out=bvb_sb[:], in_=bv_bcast)

                # ===== phase 1 interleaved with XkT + mask loads =====
                def emit_xkt(et):
                    eng = nc.sync if et % 2 == 0 else nc.scalar
                    eng.dma_start(out=XkT_sb[:, et, :], in_=XkT_d[:, et, :])

                for e2t in range(EC):
                    # phase 1: tT = (Xq @ Wqk + kb)^T  [e2, l] fp8
                    for lc in range(2):
                        ps = ps_mm.tile([P, 512], F32, name="ps1",
                                        tag="mm")
                        for e1t in range(EC):
                            nc.tensor.matmul(
                                ps[:],
                                wqk_sb[:, e1t, e2t * P:(e2t + 1) * P],
                                xqT_sb[:, e1t, lc * 512:(lc + 1) * 512],
                                start=(e1t == 0), stop=(e1t == EC - 1),
                            )
                        nc.scalar.activation(
                            out=tT_sb[:, e2t, lc * 512:(lc + 1) * 512],
                            in_=ps[:], func=AF.Identity,
                            bias=kb_sb[:, e2t:e2t + 1],
                        )
                    emit_xkt(e2t)

            def emit_stage_c():
                # ===== stage C: WvT [d, do] bf16 direct loads =====
                for dt in range(EC):
                    eng = nc.sync if dt % 2 == 0 else nc.scalar
                    eng.dma_start(out=WvT_sb[:, dt, :], in_=WvT_d[:, dt, :])

            def emit_stage_d():
                # ===== stage D: Vb = Xv natural [j, d] (bf16 from host) ====
                for jt in range(JC):
                    eng = nc.sync if jt % 2 == 0 else nc.scalar
                    eng.dma_start(out=Vb_sb[:, jt, :], in_=Xv_d[:, jt, :])

            def emit_masks():
                for lt in range(LT):
                    eng = nc.sync if lt % 2 == 0 else nc.scalar
                    eng.dma_start(out=mask_sb[:, lt, :], in_=Mk_d[:, lt, :])

            WvT_sb = lp_pool("WvT_p").tile([P, EC, D], BF16, name="WvT_sb")
            Vb_sb = lp_pool("Vb_p").tile([P, JC, D], BF16, name="Vb_sb")

            # ===== main loop pools =====
            ppool = lp_pool("pp")
            pmpool = lp_pool("pmp")
            ptpool = lp_pool("ptp")
            dnp = lp_pool("dn")

            def emit_scores(lt):
                # phase 2 (fp8 DoubleRow) + exp -> p_sb bf16 [P, J]
                p_sb = ppool.tile([P, J], BF16, name="p_sb", tag="p",
                                  bufs=4)
                for jt4 in range(4):
                    ps = ps_s.tile([P, 512], F32, name="ps_sc", tag="s",
                                   bufs=3)
                    for e2p in range(EC // 2):
                        nc.tensor.matmul(
                            ps[:],
                            tT_sb[:, 2 * e2p:2 * e2p + 2,
                                  lt * P:(lt + 1) * P],
                            XkT_sb[:, 2 * e2p:2 * e2p + 2,
                                   jt4 * 512:(jt4 + 1) * 512],
                            start=(e2p == 0), stop=(e2p == EC // 2 - 1),
                            perf_mode=DR,
                        )
                    nc.scalar.activation(
                        out=p_sb[:, jt4 * 512:(jt4 + 1) * 512],
                        in_=ps[:], func=AF.Exp, scale=SCALE,
                    )
                return p_sb

            def emit_soft(lt, p_sb):
                # pm = p * mask (unnormalized), accumulate denom; rden
                denom = dnp.tile([P, 1], F32, name="denom", tag="dn",
                                 bufs=4)
                pm = pmpool.tile([P, J], BF16, name="pm", tag="pm", bufs=4)
                nc.vector.scalar_tensor_tensor(
                    out=pm[:], in0=p_sb[:], scalar=1.0,
                    in1=mask_sb[:, lt, :],
                    op0=ALU.mult, op1=ALU.mult, accum_out=denom[:],
                )
                rden = dnp.tile([P, 1], F32, name="rden", tag="rd",
                                bufs=4)
                nc.vector.reciprocal(out=rden[:], in_=denom[:])
                return pm, rden

            def emit_pair_front(lpair):
                lts = [2 * lpair, 2 * lpair + 1]
                p_sbs = [emit_scores(lt) for lt in lts]
                return [emit_soft(lt, p_sb)
                        for lt, p_sb in zip(lts, p_sbs)]

            def emit_tr(lpair, front):
                pT_sb = ptpool.tile([P, JC, 2 * P], BF16, name="pT_sb",
                                    tag="pt", bufs=2)
                for lh in range(2):
                    pm, _ = front[lh]
                    _transpose_chunks(
                        nc, ps_tr, pm[:],
                        lambda jt, lh=lh: pT_sb[:, jt, lh * P:(lh + 1) * P],
                        JC, ident_b, BF16, "ph",
                    )
                return pT_sb

            def emit_back(lpair, pT_sb, front):
                # phase 4: zT [d, l-pair] = Xv^T p^T  (bf16)
                # last pair: split by l-half so the tail drains sooner
                zT_sb = ztpool.tile([P, EC, 2 * P], BF16, name="zT_sb",
                                    tag="zt", bufs=2)
                halves = ([(0, 2 * P)] if lpair < NP - 1
                          else [(0, P), (P, 2 * P)])
                for h0, h1 in halves:
                    for dt in range(EC):
                        ps = ps_mm.tile([P, 512], F32, name="ps4",
                                        tag="mm")
                        for jt in range(JC):
                            nc.tensor.matmul(
                                ps[:, 0:h1 - h0],
                                Vb_sb[:, jt, dt * P:(dt + 1) * P],
                                pT_sb[:, jt, h0:h1],
                                start=(jt == 0), stop=(jt == JC - 1),
                            )
                        nc.scalar.activation(out=zT_sb[:, dt, h0:h1],
                                             in_=ps[:, 0:h1 - h0],
                                             func=AF.Copy)

                    # phase 5: out = (zT^T WvT) * rden + bv
                    for lh in range(2):
                        if not (h0 <= lh * P < h1):
                            continue
                        lt = 2 * lpair + lh
                        rden = front[lh][1]
                        o_sb = opool.tile([P, D], F32, name="o_sb", tag="o",
                                          bufs=3)
                        for doc in range(2):
                            ps = ps_mm.tile([P, 512], F32, name="ps5",
                                            tag="mm")
                            for dt in range(EC):
                                nc.tensor.matmul(
                                    ps[:],
                                    zT_sb[:, dt, lh * P:(lh + 1) * P],
                                    WvT_sb[:, dt, doc * 512:(doc + 1) * 512],
                                    start=(dt == 0), stop=(dt == EC - 1),
                                )
                            nc.vector.scalar_tensor_tensor(
                                out=o_sb[:, doc * 512:(doc + 1) * 512],
                                in0=ps[:], scalar=rden[:],
                                in1=bvb_sb[:, doc * 512:(doc + 1) * 512],
                                op0=ALU.mult, op1=ALU.add,
                            )
                        eng = nc.sync if lt % 2 == 0 else nc.scalar
                        eng.dma_start(out=out_d[:, lt, :], in_=o_sb[:])

            # ===== main software pipeline =====
            emit_masks()
            front = emit_pair_front(0)
            emit_stage_d()
            emit_stage_c()
            ztpool = lp_pool("ztp")
            opool = lp_pool("op")
            pT = emit_tr(0, front)
            for lpair in range(NP):
                nxt = None
                if lpair < NP - 1:
                    nxt = emit_pair_front(lpair + 1)
                emit_back(lpair, pT, front)
                if nxt is not None:
                    front = nxt
                    pT = emit_tr(lpair + 1, front)

    nc.compile()
    return nc


_NC_CACHE = {}


def _get_nc():
    if "nc" not in _NC_CACHE:
        _NC_CACHE["nc"] = _build()
    return _NC_CACHE["nc"]


def _shard_inputs(Q, K, V, mask, Wq_w, Wq_b, Wk_w, Wk_b, Wv_w, Wv_b):
    import ml_dtypes
    bf16 = ml_dtypes.bfloat16
    fp8 = ml_dtypes.float8_e4m3
    f32 = np.float32
    Wq32 = np.asarray(Wq_w, f32)
    Wk32 = np.asarray(Wk_w, f32)
    # NOTE: the 1/sqrt(D) score scale is applied at the Exp activation
    # (scale=1/32), so Wqk/kb are unscaled here — keeps tT in fp8's
    # normal range (sigma ~ 0.33).
    common = {
        "Wqk": np.ascontiguousarray(
            (Wq32.T @ Wk32).astype(bf16)),
        "kb": np.ascontiguousarray(
            Wk32.T @ np.asarray(Wq_b, f32), f32),
        "WvT": np.ascontiguousarray(np.asarray(Wv_w, f32).astype(bf16).T),
        "bv": np.ascontiguousarray(Wv_b, f32),
    }
    in_maps = []
    for c in range(NCORES):
        b, h = divmod(c, 2)
        sl = slice(h * LS, (h + 1) * LS)
        in_maps.append({
            "XqT": np.ascontiguousarray(
                np.asarray(Q[b, sl, :], f32).astype(bf16).T),
            "XkT": np.ascontiguousarray(
                np.asarray(K[b], f32).astype(fp8).T),
            "Xv": np.ascontiguousarray(np.asarray(V[b], f32).astype(bf16)),
            "mask": np.ascontiguousarray(
                np.asarray(mask[b, sl, :]).astype(bf16)),
            **common,
        })
    return in_maps


def _run(inputs, trace=False):
    nc = _get_nc()
    in_maps = _shard_inputs(**inputs)
    res = run_bass_kernel_spmd(nc, in_maps, core_ids=list(range(NCORES)),
                               trace=trace)
    out = np.empty((B, L, D), np.float32)
    for c in range(NCORES):
        b, h = divmod(c, 2)
        out[b, h * LS:(h + 1) * LS, :] = res.results[c]["out"]
    return out, res


def kernel(**inputs):
    out, _ = _run(inputs, trace=False)
    return out
